# revision 6
# baseline (speedup 1.0000x reference)
"""2-layer GCN (GCNConv -> ReLU -> GCNConv) on 8 Trainium2 NeuronCores.

Math: gcn_conv(x, W, b) = D^-1/2 (A + I) D^-1/2 (x W) + b, where deg is the
in-degree (dst) including self-loops.  The symmetric norm factorizes:
norm(src,dst) = dinv[src]*dinv[dst], so with y' = dinv * (x@W):
    conv = dinv * (sum_{src->dst} y'[src] + y'[dst]) + b
i.e. propagation is an UNWEIGHTED gather-sum of pre-scaled rows (self-loop
is just one more gathered row), followed by a per-row scale.

Device plan (3 SPMD launches over 8 cores, nodes sharded 12544/core):
  L1: y' = dinv * (x @ W1)          (dense matmul, contiguous loads)
  L2: h' = dinv * relu(dinv * gather_sum(y') + b1)
  L3: out = (dinv * gather_sum(h')) @ W2 + b2
The gathers use gpsimd indirect DMA: per 128x7-node chunk, one indirect
DMA pulls K rows (64B each) per node from the DRAM table into SBUF
[128, K*7*16] (k-major), then a log2(K) tree of contiguous DVE adds
reduces over K.  Host pre-sorts nodes by in-degree so each chunk's K is
tight (pad slots point at a guaranteed-zero row).
"""

import os
import sys

for _p in ("/opt/trn_rl_repo", "/root/.axon_site/_ro/trn_rl_repo"):
    if os.path.isdir(_p) and _p not in sys.path:
        sys.path.append(_p)

import numpy as np

import concourse.bass as bass
import concourse.bacc as bacc
import concourse.tile as tile
from concourse import mybir
from concourse.bass_utils import run_bass_kernel_spmd
from concourse.masks import make_identity

dt = mybir.dt
F32 = dt.float32
I32 = dt.int32
ALU = mybir.AluOpType

N = 100000          # real nodes
F = 256             # input features
H = 16              # hidden
O = 40              # classes
NCORES = 8
P = 128
C = 7               # node columns per partition per chunk
NODES_PER_CHUNK = P * C          # 896 per core per chunk
CHUNKS = 14
PC = NODES_PER_CHUNK * CHUNKS    # 12544 nodes per core
NPAD = PC * NCORES               # 100352 padded node space
ZR = N                           # any row >= N is all-zero (padding rows)

_TRACE = bool(os.environ.get("GNN_TRACE"))
_EXEC_NS = []   # per-launch exec_time_ns when tracing


# --------------------------------------------------------------------------
# device programs
# --------------------------------------------------------------------------

def build_l1():
    """y' = dinv * (x @ W1) for this core's 12544 contiguous rows."""
    nc = bacc.Bacc()
    xT = nc.declare_dram_parameter("xT", [F, PC], F32, isOutput=False)
    w1 = nc.declare_dram_parameter("w1", [F, H], F32, isOutput=False)
    dinv = nc.declare_dram_parameter("dinv", [PC], F32, isOutput=False)
    yp = nc.declare_dram_parameter("yp", [PC, H], F32, isOutput=True)

    with tile.TileContext(nc) as tc:
        with (
            tc.tile_pool(name="w", bufs=1) as wp,
            tc.tile_pool(name="x", bufs=3) as xp,
            tc.tile_pool(name="d", bufs=2) as dp,
            tc.tile_pool(name="y", bufs=3) as yo,
            tc.tile_pool(name="ps", bufs=4, space="PSUM") as pp,
        ):
            w1a = wp.tile([P, H], F32, tag="w1a")
            w1b = wp.tile([P, H], F32, tag="w1b")
            nc.sync.dma_start(out=w1a[:], in_=w1[0:P, :])
            nc.sync.dma_start(out=w1b[:], in_=w1[P:F, :])

            for s in range(CHUNKS):
                cols = slice(s * NODES_PER_CHUNK, (s + 1) * NODES_PER_CHUNK)
                xa = xp.tile([P, NODES_PER_CHUNK], F32, tag="xa")
                xb = xp.tile([P, NODES_PER_CHUNK], F32, tag="xb")
                nc.sync.dma_start(out=xa[:], in_=xT[0:P, cols])
                nc.sync.dma_start(out=xb[:], in_=xT[P:F, cols])
                # dinv for rows s*896 + t*128 + p  -> [p, t]
                dv = dp.tile([P, C], F32, tag="dv")
                nc.sync.dma_start(
                    out=dv[:],
                    in_=dinv[cols].rearrange("(t p) -> p t", p=P),
                )
                yt = yo.tile([P, C * H], F32, tag="yt")
                for t in range(C):
                    ps = pp.tile([P, H], F32, tag="ps")
                    nc.tensor.matmul(
                        out=ps[:], lhsT=xa[:, t * P:(t + 1) * P], rhs=w1a[:],
                        start=True, stop=False,
                    )
                    nc.tensor.matmul(
                        out=ps[:], lhsT=xb[:, t * P:(t + 1) * P], rhs=w1b[:],
                        start=False, stop=True,
                    )
                    nc.vector.tensor_scalar(
                        out=yt[:, t * H:(t + 1) * H], in0=ps[:],
                        scalar1=dv[:, t:t + 1], scalar2=None, op0=ALU.mult,
                    )
                # rows s*896 + t*128 + p  -> [p, (t h)]
                nc.sync.dma_start(
                    out=yp[cols, :].rearrange("(t p) h -> p t h", p=P),
                    in_=yt[:].rearrange("p (t h) -> p t h", h=H),
                )
    nc.compile()
    return nc


def _gather_sum(nc, tc, pools, table, idx, ks, chunk, out_pool):
    """Gather ks[chunk] rows/node for 896 nodes, tree-sum over k.

    Returns an SBUF AP [128, 112] = [p, (c 16)] with the per-node sums.
    """
    ip, gp = pools
    K = ks[chunk]
    off = int(np.sum([k * NODES_PER_CHUNK for k in ks[:chunk]]))
    nidx = P * K * C
    idxt = ip.tile([P, K * C], I32, tag="idxt")
    nc.sync.dma_start(
        out=idxt[:],
        in_=idx[off:off + nidx].rearrange("(p q) -> p q", p=P),
    )
    g = gp.tile([P, K * C * H], F32, tag="g")
    # HW consumes ONE offset per partition per indirect DMA: each call
    # gathers 128 rows (64B each), one slot column at a time.
    for q in range(K * C):
        nc.gpsimd.indirect_dma_start(
            out=g[:, q * H:(q + 1) * H],
            out_offset=None,
            in_=table[:],
            in_offset=bass.IndirectOffsetOnAxis(ap=idxt[:, q:q + 1], axis=0),
        )
    k = K
    CH = C * H
    while k > 1:
        half = k // 2
        nc.vector.tensor_tensor(
            out=g[:, 0:half * CH],
            in0=g[:, 0:half * CH],
            in1=g[:, (k - half) * CH:k * CH],
            op=ALU.add,
        )
        k -= half
    return g


def build_l2(ks):
    """h' = dinv * relu(dinv * gather_sum(y') + b1) over permuted layout."""
    stot = int(np.sum([k * NODES_PER_CHUNK for k in ks]))
    nc = bacc.Bacc()
    table = nc.declare_dram_parameter("table", [NPAD, H], F32, isOutput=False)
    idx = nc.declare_dram_parameter("idx", [stot], I32, isOutput=False)
    dinv = nc.declare_dram_parameter("dinv", [PC], F32, isOutput=False)
    b1t = nc.declare_dram_parameter("b1t", [P, H], F32, isOutput=False)
    hp = nc.declare_dram_parameter("hp", [PC, H], F32, isOutput=True)

    with tile.TileContext(nc) as tc:
        with (
            tc.tile_pool(name="cst", bufs=1) as cp,
            tc.tile_pool(name="ip", bufs=2) as ip,
            tc.tile_pool(name="gp", bufs=2) as gp,
            tc.tile_pool(name="dp", bufs=2) as dp,
            tc.tile_pool(name="hp", bufs=3) as ho,
        ):
            b1s = cp.tile([P, H], F32, tag="b1s")
            nc.sync.dma_start(out=b1s[:], in_=b1t[:, :])
            b1b = b1s[:].unsqueeze(1).to_broadcast([P, C, H])

            for ch in range(CHUNKS):
                g = _gather_sum(nc, tc, (ip, gp), table, idx, ks, ch, ho)
                rows = slice(ch * NODES_PER_CHUNK, (ch + 1) * NODES_PER_CHUNK)
                dv = dp.tile([P, C], F32, tag="dv")
                nc.sync.dma_start(
                    out=dv[:], in_=dinv[rows].rearrange("(p c) -> p c", p=P)
                )
                dvb = dv[:].unsqueeze(2).to_broadcast([P, C, H])
                s3 = g[:, 0:C * H].rearrange("p (c h) -> p c h", h=H)
                h = ho.tile([P, C * H], F32, tag="h")
                h3 = h[:].rearrange("p (c h) -> p c h", h=H)
                nc.vector.tensor_tensor(out=h3, in0=s3, in1=dvb, op=ALU.mult)
                nc.vector.tensor_tensor(out=h3, in0=h3, in1=b1b, op=ALU.add)
                nc.vector.tensor_scalar_max(out=h[:], in0=h[:], scalar1=0.0)
                nc.vector.tensor_tensor(out=h3, in0=h3, in1=dvb, op=ALU.mult)
                nc.sync.dma_start(
                    out=hp[rows, :].rearrange("(p c) h -> p c h", p=P),
                    in_=h[:].rearrange("p (c h) -> p c h", h=H),
                )
    nc.compile()
    return nc


def build_l3(ks):
    """out = (dinv * gather_sum(h')) @ W2 + b2 over permuted layout."""
    stot = int(np.sum([k * NODES_PER_CHUNK for k in ks]))
    nc = bacc.Bacc()
    table = nc.declare_dram_parameter("table", [NPAD, H], F32, isOutput=False)
    idx = nc.declare_dram_parameter("idx", [stot], I32, isOutput=False)
    dinv = nc.declare_dram_parameter("dinv", [PC], F32, isOutput=False)
    w2 = nc.declare_dram_parameter("w2", [H, O], F32, isOutput=False)
    b2t = nc.declare_dram_parameter("b2t", [P, O], F32, isOutput=False)
    out = nc.declare_dram_parameter("out", [PC, O], F32, isOutput=True)

    with tile.TileContext(nc) as tc:
        with (
            tc.tile_pool(name="cst", bufs=1) as cp,
            tc.tile_pool(name="ip", bufs=2) as ip,
            tc.tile_pool(name="gp", bufs=2) as gp,
            tc.tile_pool(name="dp", bufs=2) as dp,
            tc.tile_pool(name="go", bufs=2) as go,
            tc.tile_pool(name="tp", bufs=3) as tp,
            tc.tile_pool(name="oo", bufs=3) as oo,
            tc.tile_pool(name="pst", bufs=4, space="PSUM") as pst,
            tc.tile_pool(name="pso", bufs=4, space="PSUM") as pso,
        ):
            w2s = cp.tile([H, O], F32, tag="w2s")
            nc.sync.dma_start(out=w2s[:], in_=w2[:, :])
            b2s = cp.tile([P, O], F32, tag="b2s")
            nc.sync.dma_start(out=b2s[:], in_=b2t[:, :])
            ident = cp.tile([P, P], F32, tag="ident")
            make_identity(nc, ident[:])

            for ch in range(CHUNKS):
                g = _gather_sum(nc, tc, (ip, gp), table, idx, ks, ch, go)
                rows = slice(ch * NODES_PER_CHUNK, (ch + 1) * NODES_PER_CHUNK)
                dv = dp.tile([P, C], F32, tag="dv")
                nc.sync.dma_start(
                    out=dv[:], in_=dinv[rows].rearrange("(p c) -> p c", p=P)
                )
                dvb = dv[:].unsqueeze(2).to_broadcast([P, C, H])
                s3 = g[:, 0:C * H].rearrange("p (c h) -> p c h", h=H)
                gs = go.tile([P, C * H], F32, tag="gs")
                gs3 = gs[:].rearrange("p (c h) -> p c h", h=H)
                nc.vector.tensor_tensor(out=gs3, in0=s3, in1=dvb, op=ALU.mult)

                ot = oo.tile([P, C * O], F32, tag="ot")
                for t in range(C):
                    gT_ps = pst.tile([H, P], F32, tag="gT_ps")
                    nc.tensor.transpose(
                        out=gT_ps[:], in_=gs[:, t * H:(t + 1) * H],
                        identity=ident[:],
                    )
                    gT = tp.tile([H, P], F32, tag="gT")
                    nc.vector.tensor_copy(out=gT[:], in_=gT_ps[:])
                    o_ps = pso.tile([P, O], F32, tag="o_ps")
                    nc.tensor.matmul(
                        out=o_ps[:], lhsT=gT[:], rhs=w2s[:],
                        start=True, stop=True,
                    )
                    nc.vector.tensor_tensor(
                        out=ot[:, t * O:(t + 1) * O], in0=o_ps[:], in1=b2s[:],
                        op=ALU.add,
                    )
                nc.sync.dma_start(
                    out=out[rows, :].rearrange("(p c) o -> p c o", p=P),
                    in_=ot[:].rearrange("p (c o) -> p c o", o=O),
                )
    nc.compile()
    return nc


# --------------------------------------------------------------------------
# host orchestration
# --------------------------------------------------------------------------

def _install_trace_shim():
    """Provide antenv.axon_hooks (missing in this image) so bass_utils can
    NTFF-profile under axon, and neuter the artifact upload."""
    import types
    import contextlib
    import ctypes

    if "antenv.axon_hooks" not in sys.modules:
        lib = ctypes.CDLL("/opt/axon/libaxon_pjrt.so")
        lib.axon_start_nrt_profile.argtypes = [
            ctypes.POINTER(ctypes.c_int64), ctypes.c_size_t]
        lib.axon_start_nrt_profile.restype = ctypes.c_int64
        lib.axon_stop_nrt_profile.argtypes = [ctypes.c_char_p]
        lib.axon_stop_nrt_profile.restype = ctypes.c_int64

        @contextlib.contextmanager
        def _hook(output_dir, device_ids):
            import jax
            jax.devices()
            if device_ids:
                ids = (ctypes.c_int64 * len(device_ids))(*device_ids)
                rc = lib.axon_start_nrt_profile(ids, len(device_ids))
            else:
                rc = lib.axon_start_nrt_profile(None, 0)
            if rc != 0:
                raise RuntimeError(f"axon_start_nrt_profile rc={rc}")
            try:
                yield
            finally:
                n = lib.axon_stop_nrt_profile(str(output_dir).encode())
                print(f"profile: {n} file(s) -> {output_dir}", file=sys.stderr)

        mod = types.ModuleType("antenv.axon_hooks")
        mod.get_axon_ntff_profile_hook = lambda: _hook
        mod.set_axon_ntff_profile_hook = lambda h: None
        sys.modules["antenv.axon_hooks"] = mod

    import concourse.bass_utils as bu
    bu.upload_artifacts = lambda tmpdir: "local://skipped"


def _run(nc, in_maps, label):
    if _TRACE:
        _install_trace_shim()
        res = run_bass_kernel_spmd(
            nc, in_maps, list(range(NCORES)), trace=True, trace_cores=[0],
        )
        print(f"[{label}] exec_time_ns={res.exec_time_ns}", file=sys.stderr)
        _EXEC_NS.append((label, res.exec_time_ns))
        if res.instructions_and_trace is not None:
            print(f"[{label}] trace={res.instructions_and_trace[1]}",
                  file=sys.stderr)
        return res.results
    return run_bass_kernel_spmd(nc, in_maps, list(range(NCORES))).results


def kernel(x, edge_index, W1, b1, W2, b2):
    x = np.ascontiguousarray(np.asarray(x, dtype=np.float32))
    ei = np.asarray(edge_index)
    W1 = np.ascontiguousarray(np.asarray(W1, dtype=np.float32))
    b1 = np.asarray(b1, dtype=np.float32).reshape(-1)
    W2 = np.ascontiguousarray(np.asarray(W2, dtype=np.float32))
    b2 = np.asarray(b2, dtype=np.float32).reshape(-1)
    src = np.ascontiguousarray(ei[0]).astype(np.int64)
    dst = np.ascontiguousarray(ei[1]).astype(np.int64)
    E = src.shape[0]

    # degrees / normalization (deg counts dst occurrences + self-loop)
    counts = np.bincount(dst, minlength=NPAD).astype(np.int64)  # in-deg, no self
    dinv = np.zeros(NPAD, np.float32)
    dinv[:N] = 1.0 / np.sqrt((counts[:N] + 1).astype(np.float64))

    # CSR of in-edges, sorted by dst
    order_e = np.argsort(dst, kind="stable")
    src_sorted = src[order_e].astype(np.int32)
    starts = np.zeros(NPAD + 1, np.int64)
    np.cumsum(counts, out=starts[1:])

    # node layout: sort by in-degree desc, deal round-robin to cores,
    # chunk-rows of 8*896 sorted nodes share one K
    ordern = np.argsort(-counts, kind="stable").astype(np.int64)  # [NPAD]
    blocks = ordern.reshape(CHUNKS, NODES_PER_CHUNK * NCORES)
    # node_layout[core, ch, q]  (q = p*C + col)
    node_layout = blocks.reshape(CHUNKS, NODES_PER_CHUNK, NCORES).transpose(2, 0, 1)
    ks = [int(counts[blocks[ch]].max()) + 1 for ch in range(CHUNKS)]  # +1 self

    # padded gather-index arrays, k-major: idx[p, k*C + col]
    idx_cores = []
    for core in range(NCORES):
        parts = []
        for ch in range(CHUNKS):
            nodes = node_layout[core, ch]          # [896] in q order
            K = ks[ch]
            kk = np.arange(K, dtype=np.int64)
            pos = starts[nodes][:, None] + kk[None, :] - 1
            valid = (kk[None, :] >= 1) & (kk[None, :] <= counts[nodes][:, None])
            vals = np.where(
                kk[None, :] == 0,
                nodes[:, None],
                np.where(valid, src_sorted[np.clip(pos, 0, E - 1)], ZR),
            ).astype(np.int32)                     # [896, K]
            # [896, K] -> [128, C, K] -> [128, K, C] -> flat
            vals = vals.reshape(P, C, K).transpose(0, 2, 1)
            parts.append(np.ascontiguousarray(vals).reshape(-1))
        idx_cores.append(np.concatenate(parts))

    dinv_l2 = [
        np.ascontiguousarray(dinv[node_layout[core].reshape(-1)])
        for core in range(NCORES)
    ]
    layout_flat = [node_layout[core].reshape(-1) for core in range(NCORES)]

    # L1 inputs: x padded + transposed; original-order contiguous shards
    xT = np.zeros((F, NPAD), np.float32)
    xT[:, :N] = x.T
    b1t = np.ascontiguousarray(np.tile(b1[None, :], (P, 1)))
    b2t = np.ascontiguousarray(np.tile(b2[None, :], (P, 1)))

    # ---- L1 ----
    nc1 = build_l1()
    maps1 = [
        {
            "xT": np.ascontiguousarray(xT[:, core * PC:(core + 1) * PC]),
            "w1": W1,
            "dinv": np.ascontiguousarray(dinv[core * PC:(core + 1) * PC]),
        }
        for core in range(NCORES)
    ]
    r1 = _run(nc1, maps1, "L1")
    ypad = np.concatenate([r1[i]["yp"] for i in range(NCORES)], axis=0)

    # ---- L2 ----
    nc2 = build_l2(ks)
    maps2 = [
        {"table": ypad, "idx": idx_cores[core], "dinv": dinv_l2[core],
         "b1t": b1t}
        for core in range(NCORES)
    ]
    r2 = _run(nc2, maps2, "L2")
    hpad = np.zeros((NPAD, H), np.float32)
    for core in range(NCORES):
        hpad[layout_flat[core]] = r2[core]["hp"]

    # ---- L3 ----
    nc3 = build_l3(ks)
    maps3 = [
        {"table": hpad, "idx": idx_cores[core], "dinv": dinv_l2[core],
         "w2": W2, "b2t": b2t}
        for core in range(NCORES)
    ]
    r3 = _run(nc3, maps3, "L3")
    outp = np.zeros((NPAD, O), np.float32)
    for core in range(NCORES):
        outp[layout_flat[core]] = r3[core]["out"]
    return np.ascontiguousarray(outp[:N])
